# revision 39
# baseline (speedup 1.0000x reference)
"""Trainium2 Bass kernel for nn_AttentionPositionAlign.

Reference computation (per batch b):
    src = query @ Wq                    # [M, H]
    tgt = memory @ Wm                   # [N, H]
    aligns = relu(src[:,None,:] + tgt[None,:,:])   # [M, N, H]
    out = aligns.reshape(M, N*H) @ Wout # [M, 4]

Strategy: data-parallel over B across the 8 NeuronCores (B == 8). All
compute happens in "transposed land" (H on SBUF partitions, M on the free
dim) so the Bahdanau broadcast-add becomes a per-partition scalar bias
that fuses into a single elementwise pass — the [B,M,N,H] intermediate
(604 MB) is never materialized:

    srcT[h, m] = (Wq.T @ query.T)[h, m]         PSUM-accumulated matmuls
    tgtT[h, n] = (Wm.T @ memory.T)[h, n]        direct-orientation matmuls
                                                (FD=36, FWL weight loads)
    for each (hc, n) chunk c (N*H/128 = 144 of them):
        Rt = relu(srcT[hc] + tgtT[hc][:, n])    ONE fused op per chunk:
                                                DVE tensor_scalar(add,max)
                                                or ACT activation(Relu,bias)
        psum_out[32g+k, m] += Wout_c.T @ Rt     col-tiled (tile_position)
                                                matmuls, 4 concurrent PE
                                                column groups, 144-deep
                                                PSUM accumulation
    out[k, m] = sum_g psum_out[32g+k, m]        selector matmul, then host
                                                transposes [4, M] -> [M, 4]

Measured per-op rates (HW): DVE tensor_scalar [128,1024] bf16 = 487ns
isolated, ~415-435ns sustained (4x-mode marginal rate, independent ops
pipeline); ACT ACTIVATE = (FD+352)/1.2 ns from SBUF, (FD+311)/1.2 from
PSUM, dtype-independent, no accel modes; GpSimd is useless (14.8us/op,
no PSUM access).  So the relu stage floor is ~45us with chunks split 36
ACT / 108 DVE (reverse-Bresenham so the last chunks are DVE).  ACT
chunks read the projection PSUM directly (multi-bank [128,1024] f32
APs); only the DVE chunks need the bf16 SBUF copy of srcT (on ACT —
every other placement loses the balance race).

Latency tricks around the ~45us relu core:
 - dma_start costs ~650ns serial issue per queue; the critical set
   (wq[hc0], qT-first-half on Sync; mT, wm[hc0] on the ACT HWDGE queue)
   issues in parallel ahead of the bulk, and qT ships m-half-major so
   the mc0 projection unblocks early; the leading SPLIT_K chunks run as
   per-mc half-ops while the second qT half is in flight.
 - 9 dummy matmuls warm the PE's HAM clock gate (else the projections
   run at 1.2 instead of 2.4 GHz right when they gate the relu start).
 - tgtT is projected directly in transposed orientation (wm as lhsT,
   FD=36 matmuls) - no PE transpose, no extra ACT copies.
 - the final col-group reduce reuses drained PSUM banks and ships the
   output as per-mc DMAs.
"""

import numpy as np

import concourse.bass as bass
import concourse.tile as tile
from concourse import bacc, mybir
from concourse.bass_utils import run_bass_kernel_spmd

B, M, N, H = 8, 1024, 36, 512
DQ, DM = 512, 2048
P = 128
HC = H // P          # 4 h-chunks
DQC = DQ // P        # 4
DMC = DM // P        # 16
MC = 2               # m-chunks for 512-wide PSUM banks
MF = M // MC         # 512
NCHUNK = N * HC      # 144 contraction chunks of 128

f32 = mybir.dt.float32
f32r = mybir.dt.float32r
bf16 = mybir.dt.bfloat16

# Knobs
R_DT = bf16          # dtype of the relu output / contraction rhs+lhsT
SRC_DT = bf16        # dtype of the srcT store / relu input
IN_DT = bf16         # dtype inputs are shipped in (f32r or bf16)
N_ACT = 36           # chunks assigned to ACT (rest on DVE)
COL_TILE = 4         # concurrent PE column groups for the contraction
SPLIT_K = 12         # leading hc0 chunks emitted as per-mc half-ops

_CACHE = {}


N_ACT_LAST = NCHUNK - 8  # confine ACT chunks to the first 136: the last


def _is_act(c):
    # Bresenham spread over the first N_ACT_LAST chunks only, so the
    # final 8 chunks all land on the faster DVE and the slow ACT op
    # never gates the post-relu reduce chain.
    if c >= N_ACT_LAST:
        return False
    return (c + 1) * N_ACT // N_ACT_LAST > c * N_ACT // N_ACT_LAST


def _build():
    nc = bacc.Bacc("TRN2", target_bir_lowering=False, debug=False, num_devices=B)

    qT = nc.dram_tensor("qT", [P, DQC * M], IN_DT, kind="ExternalInput").ap()
    sel = nc.dram_tensor("sel", [P, 4], f32r, kind="ExternalInput").ap()
    mT = nc.dram_tensor("mT", [P, DMC * N], IN_DT, kind="ExternalInput").ap()
    wq = nc.dram_tensor("wq", [P, HC * DQC * P], IN_DT, kind="ExternalInput").ap()
    wm = nc.dram_tensor("wm", [P, DMC * H], IN_DT, kind="ExternalInput").ap()
    wo = nc.dram_tensor("wo", [P, NCHUNK * 4], R_DT, kind="ExternalInput").ap()
    out = nc.dram_tensor("out", [4, M], f32, kind="ExternalOutput").ap()

    with tile.TileContext(nc) as tc:
        with (
            tc.tile_pool(name="weights", bufs=1) as wpool,
            tc.tile_pool(name="acts", bufs=1) as apool,
            tc.tile_pool(name="rpool", bufs=16) as rpool,
            tc.tile_pool(name="ppool", bufs=2, space="PSUM") as ppool,
            tc.tile_pool(name="opool", bufs=1, space="PSUM") as opool,
        ):
            # --- SBUF tiles
            sel_sb = wpool.tile([P, 4], f32r)
            mT_sb = wpool.tile([P, DMC, N], IN_DT)
            wm_sb = wpool.tile([P, HC, DMC, P], IN_DT)
            wq_sb = wpool.tile([P, HC, DQC, P], IN_DT)
            qT_sb = wpool.tile([P, MC, DQC, MF], IN_DT)
            wo_sb = wpool.tile([P, NCHUNK * 4], R_DT)

            # --- DMA priority order (~650ns serial issue each, FIFO-ish
            # transfers): the critical set in need-order — wq[hc0], the qT
            # m-half that feeds the mc0 projection, then mT+wm[hc0] for the
            # tgt biases (shortest post-arrival latency last), bulk after.
            # qT is packed m-half-major so its first half unblocks the mc0
            # projection (and the leading half-chunks) ~3us earlier.
            nc.sync.dma_start(wq_sb[:, 0, :, :], wq[:, : DQC * P])
            nc.scalar.dma_start(mT_sb[:], mT[:])
            nc.scalar.dma_start(wm_sb[:, 0, :, :], wm[:, : DMC * P])
            for mc in range(MC):
                nc.sync.dma_start(
                    qT_sb[:, mc, :, :],
                    qT[:, mc * DQC * MF : (mc + 1) * DQC * MF],
                )
            nc.sync.dma_start(wo_sb[:], wo[:])
            nc.sync.dma_start(sel_sb[:], sel[:])
            nc.sync.dma_start(wq_sb[:, 1:, :, :], wq[:, DQC * P :])
            nc.sync.dma_start(
                wm_sb[:, 1:, :, :], wm[:, DMC * P :]
            )

            # --- PE warm-up: the HAM clock gate holds the PE at 1.2 GHz
            # until it has been busy ~3.4us. The PE would otherwise idle
            # through the DMA window and run the latency-critical
            # projections cold. ~9 dummy matmuls into a scratch PSUM bank
            # (reused as `ro` in the tail) flip it to 2.4 GHz for free — and
            # they must END close enough to the qT-gated projections that
            # the ~3.4us MID idle window doesn't re-throttle in between.
            po = [opool.tile([P, MF], f32, name=f"po{mc}") for mc in range(MC)]
            warm = opool.tile([P, MF], f32, name="warm")
            zw = wpool.tile([P, MF], R_DT)
            nc.vector.memset(zw[:], 0.0)
            for _ in range(10):
                nc.tensor.matmul(
                    warm[:], zw[:, :P], zw[:],
                    start=True, stop=True, skip_group_check=True,
                )
            # zero-init po (sets has_written on all 128 partitions so
            # col-group matmuls can accumulate)
            for mc in range(MC):
                nc.tensor.matmul(
                    po[mc][:], zw[:, :P], zw[:],
                    start=True, stop=False, skip_group_check=True,
                )

            srcT_sb = apool.tile([P, HC, M], SRC_DT)
            tgt_sb = apool.tile([P, HC, N], f32)

            def proj_mm(hc):
                # srcT[hc] first (the long pole: its SBUF copy gates the
                # DVE chunks): one 2-bank PSUM tile; ACT chunks read it
                # in place, DVE chunks read the bf16 SBUF copy.
                ps = ppool.tile([P, M], f32, tag="proj")
                for mc in range(MC):
                    for dq in range(DQC):
                        nc.tensor.matmul(
                            ps[:, mc * MF : (mc + 1) * MF],
                            wq_sb[:, hc, dq, :],
                            qT_sb[:, mc, dq, :],
                            start=(dq == 0),
                            stop=(dq == DQC - 1),
                        )
                # tgtT[hc] directly: out[h, n] += wm[dm]^T @ mT[dm]
                pz = opool.tile([P, N], f32, tag="tproj")
                for dm in range(DMC):
                    nc.tensor.matmul(
                        pz[:],
                        wm_sb[:, hc, dm, :],
                        mT_sb[:, dm, :],
                        start=(dm == 0),
                        stop=(dm == DMC - 1),
                    )
                return pz, ps

            def proj_copy(hc, pz, ps):
                # (GPSIMD cannot access PSUM, so the srcT copy stays on ACT)
                nc.vector.tensor_copy(tgt_sb[:, hc, :], pz[:])
                nc.scalar.copy(srcT_sb[:, hc, :], ps[:])

            def proj_copy_half(hc, ps, mc):
                nc.scalar.copy(
                    srcT_sb[:, hc, mc * MF : (mc + 1) * MF],
                    ps[:, mc * MF : (mc + 1) * MF],
                )

            def chunks(hc, ps, mid_emit=None, split=0):
                rs = {}

                def ops(n, mclo, mchi):
                    c = hc * N + n
                    if n in rs:
                        r = rs[n]
                    else:
                        r = rpool.tile([P, M], R_DT)
                        rs[n] = r
                    bias = tgt_sb[:, hc, n : n + 1]
                    sl = slice(mclo * MF, mchi * MF)
                    if _is_act(c):
                        nc.scalar.activation(
                            r[:, sl],
                            ps[:, sl],
                            mybir.ActivationFunctionType.Relu,
                            bias=bias,
                            scale=1.0,
                        )
                    else:
                        nc.vector.tensor_scalar(
                            r[:, sl],
                            srcT_sb[:, hc, sl],
                            bias,
                            0.0,
                            mybir.AluOpType.add,
                            mybir.AluOpType.max,
                        )
                    g = c % COL_TILE
                    for mc in range(mclo, mchi):
                        nc.tensor.matmul(
                            po[mc][32 * g : 32 * g + 4, :],
                            wo_sb[:, 4 * c : 4 * c + 4],
                            r[:, mc * MF : (mc + 1) * MF],
                            start=False,
                            stop=(c >= NCHUNK - COL_TILE),
                            tile_position=(0, 32 * g),
                            skip_group_check=True,
                        )

                for n in range(N):
                    if n == 6 and mid_emit is not None:
                        mid_emit()
                    if n < split:
                        # mc0 half only — runs while the qT mc1 half is
                        # still in flight
                        ops(n, 0, 1)
                        if n == 2:
                            proj_copy_half(hc, ps, 1)
                        if n == split - 1:
                            for n2 in range(split):
                                ops(n2, 1, MC)
                    else:
                        ops(n, 0, MC)

            # Software-pipelined emission: hc+1's projection matmuls enter
            # the PE stream BEFORE hc's contraction matmuls; hc+1's
            # PSUM->SBUF copies are emitted a few chunks into hc's relu
            # stream so they don't head-block the ACT queue.
            prev = proj_mm(0)
            nc.vector.tensor_copy(tgt_sb[:, 0, :], prev[0][:])
            proj_copy_half(0, prev[1], 0)
            for hc in range(HC):
                if hc + 1 < HC:
                    nxt = proj_mm(hc + 1)
                    chunks(
                        hc, prev[1],
                        mid_emit=lambda h=hc + 1, t=nxt: proj_copy(h, *t),
                        split=SPLIT_K if hc == 0 else 0,
                    )
                    prev = nxt
                else:
                    chunks(hc, prev[1])

            # --- cross-partition reduce of the 4 column groups via a
            # selector matmul: out[k, m] = sum_g po[32g+k, m]; per-mc
            # output DMAs overlap the ~1.3us DMA ramp.
            out_sb = apool.tile([4, M], f32)
            for mc in range(MC):
                # the two PSUM->SBUF casts run concurrently (mc0 on DVE,
                # mc1 on the by-now-idle ACT)
                pf = apool.tile([P, MF], f32r, name=f"pf{mc}")
                if mc == 0:
                    nc.vector.tensor_copy(pf[:], po[mc][:])
                else:
                    nc.scalar.copy(pf[:], po[mc][:])
                # reuse scratch PSUM (warm for mc0, drained po[0] for mc1)
                # so the two reduces don't WAR-serialize on one bank
                ro = warm[:4, :] if mc == 0 else po[0][:4, :]
                nc.tensor.matmul(
                    ro, sel_sb[:], pf[:],
                    start=True, stop=True, skip_group_check=True,
                )
                nc.vector.tensor_copy(out_sb[:, mc * MF : (mc + 1) * MF], ro)
                nc.sync.dma_start(
                    out[:, mc * MF : (mc + 1) * MF],
                    out_sb[:, mc * MF : (mc + 1) * MF],
                    single_packet=True,
                )

    nc.compile()
    return nc


def _sel_array():
    s = np.zeros((P, 4), np.float32)
    for p in range(P):
        if p % 32 < 4:
            s[p, p % 32] = 1.0
    return s


def _np_in_dt():
    if IN_DT == bf16:
        import ml_dtypes

        return ml_dtypes.bfloat16
    return np.float32


def _pack_partition_major(a, chunks):
    """[chunks*128, X] -> [128, chunks*X] with chunk-major free dim."""
    x = a.shape[1]
    return (
        np.ascontiguousarray(a.reshape(chunks, P, x).transpose(1, 0, 2))
        .reshape(P, chunks * x)
        .astype(_np_in_dt())
    )


def kernel(query, memory, Wq, Wm, Wout):
    if "nc" not in _CACHE:
        _CACHE["nc"] = _build()
    nc = _CACHE["nc"]
    in_maps = _make_in_maps(query, memory, Wq, Wm, Wout)
    res = run_bass_kernel_spmd(nc, in_maps, list(range(B)))
    return np.stack([res.results[b]["out"].T for b in range(B)]).astype(np.float32)


def _make_in_maps(query, memory, Wq, Wm, Wout):
    # wq packed [p, (hc, dq, 128)]: Wq[dq*128+p, hc*128+j]
    wq_p = (
        np.ascontiguousarray(
            np.asarray(Wq, np.float32).reshape(DQC, P, HC, P).transpose(1, 2, 0, 3)
        )
        .reshape(P, HC * DQC * P)
        .astype(_np_in_dt())
    )
    # wm packed [hi, (hc, dm, 128)]: Wm[dm*128+hi, hc*128+hin]
    wm_p = (
        np.ascontiguousarray(
            np.asarray(Wm, np.float32).reshape(DMC, P, HC, P).transpose(1, 2, 0, 3)
        )
        .reshape(P, DM * HC * P // P)
        .astype(_np_in_dt())
    )
    # Wout rows are n*H + hc*128 + p; kernel chunk id c = hc*N + n (hc-major)
    wo_p = np.ascontiguousarray(
        np.asarray(Wout, np.float32).reshape(N, HC, P, 4).transpose(2, 1, 0, 3)
    ).reshape(P, NCHUNK * 4)
    if R_DT == bf16:
        import ml_dtypes

        wo_p = wo_p.astype(ml_dtypes.bfloat16)
    in_maps = []
    for b in range(B):
        # qT packed [p, (mh, dq, 512)]: queryT[dq*128+p, mh*512+j]
        qT_p = (
            np.ascontiguousarray(
                np.asarray(query[b], np.float32)
                .T.reshape(DQC, P, MC, MF)
                .transpose(1, 2, 0, 3)
            )
            .reshape(P, MC * DQC * MF)
            .astype(_np_in_dt())
        )
        mT_p = _pack_partition_major(
            np.ascontiguousarray(np.asarray(memory[b], np.float32).T), DMC
        )
        m = {
            "qT": qT_p,
            "mT": mT_p,
            "wq": wq_p,
            "wm": wm_p,
            "wo": wo_p,
            "sel": _sel_array(),
        }
        in_maps.append(m)
    return in_maps


def bench(inputs, iters=20):
    """Time repeated executions of the compiled kernel with inputs resident
    on device. Returns a list of per-call wall seconds."""
    import time

    import jax
    from jax.sharding import Mesh, PartitionSpec
    from jax.experimental.shard_map import shard_map

    from concourse import bass2jax, mybir as _mybir

    if "nc" not in _CACHE:
        _CACHE["nc"] = _build()
    nc = _CACHE["nc"]
    in_maps = _make_in_maps(**inputs)

    bass2jax.install_neuronx_cc_hook()
    partition_name = nc.partition_id_tensor.name if nc.partition_id_tensor else None
    in_names, out_names, out_avals, zero_outs = [], [], [], []
    for alloc in nc.m.functions[0].allocations:
        if not isinstance(alloc, _mybir.MemoryLocationSet):
            continue
        name = alloc.memorylocations[0].name
        if alloc.kind == "ExternalInput":
            if name != partition_name:
                in_names.append(name)
        elif alloc.kind == "ExternalOutput":
            shape = tuple(alloc.tensor_shape)
            dtype = _mybir.dt.np(alloc.dtype)
            out_names.append(name)
            out_avals.append(jax.core.ShapedArray(shape, dtype))
            zero_outs.append(np.zeros(shape, dtype))
    n_params = len(in_names)
    n_outs = len(out_avals)
    all_in_names = list(in_names) + list(out_names)
    if partition_name is not None:
        all_in_names.append(partition_name)

    def _body(*args):
        operands = list(args)
        if partition_name is not None:
            operands.append(bass2jax.partition_id_tensor())
        outs = bass2jax._bass_exec_p.bind(
            *operands,
            out_avals=tuple(out_avals),
            in_names=tuple(all_in_names),
            out_names=tuple(out_names),
            lowering_input_output_aliases=(),
            sim_require_finite=True,
            sim_require_nnan=True,
            nc=nc,
        )
        return tuple(outs)

    devices = jax.devices()[:B]
    mesh = Mesh(np.asarray(devices), ("core",))
    in_specs = (PartitionSpec("core"),) * (n_params + n_outs)
    out_specs = (PartitionSpec("core"),) * n_outs
    sharded = jax.jit(
        shard_map(
            _body, mesh=mesh, in_specs=in_specs, out_specs=out_specs, check_rep=False
        ),
        donate_argnums=tuple(range(n_params, n_params + n_outs)),
        keep_unused=True,
    )
    concat_in = [
        np.concatenate([np.asarray(in_maps[c][nm]) for c in range(B)], axis=0)
        for nm in in_names
    ]
    dev_in = [jax.device_put(a) for a in concat_in]

    def zeros():
        return [np.zeros((B * z.shape[0], *z.shape[1:]), z.dtype) for z in zero_outs]

    # warmup (compile)
    out = sharded(*dev_in, *zeros())
    jax.block_until_ready(out)

    times = []
    for _ in range(iters):
        t0 = time.perf_counter()
        out = sharded(*dev_in, *zeros())
        jax.block_until_ready(out)
        times.append(time.perf_counter() - t0)
    return times
